# revision 1
# baseline (speedup 1.0000x reference)
"""Multi-head attention (B=2, S=2048, D=1024, H=16, dk=64) on 8 Trainium2
NeuronCores via Bass/Tile.

Sharding: core c handles batch b = c//4 and head-group g = c%4 (4 heads,
256 qkv columns).  Each core computes its QKV projection slices, 4 heads of
attention, and a partial output projection against its 256-row slice of Wo.
The host sums the 4 partial outputs per batch (row-sharded Wo => partial
sums) and folds in the biases bo and bv@Wo (softmax rows sum to 1, so the
V-bias contributes exactly bv@Wo per token).

v2 design notes:
- All matmuls in float32r (full-rate fp32 PE mode, ~1.5e-4 rounding); PE
  transposes also f32r (1.5 cyc/row) to avoid dtype switches.
- scoresT [k_tok, q_tok] per head via K=64 row-packed head pairs
  (tile_position (0,0)/(64,0) derived from base partitions) -> concurrent.
- Scores land in one shared 4-bank PSUM tensor [128, 8, 512]; ONE ACT exp
  per 2 k-chunks covers [128, 2048] (amortizes the 352-cycle ACT overhead).
- AV lhsT = [1 | V_h] so PSUM row 0 accumulates the softmax denominators.
- Normalization without PE transposes: DVE reciprocal of the sums row,
  PE ones-outer-product broadcast to [65, 512], DVE multiply, then a
  partition-shifting SBUF->SBUF DMA routes each head into O^T layout.
"""

import numpy as np

P = 128
B, S, D = 2, 2048, 1024
H, DK = 16, 64
COLS = 256          # qkv columns per core (4 heads)
KC = D // P         # 8 contraction chunks for the projections
TT = 512            # token block (matmul free dim)
NJ = S // TT        # 4 token blocks
NT = S // P         # 16 token tiles
NKT = S // P        # 16 key tiles
VW = 65             # per-head AV lhsT width: ones column + 64 v-dims

_CACHE = {}


def _build():
    import concourse.bass as bass
    import concourse.tile as tile
    from concourse import bacc, mybir

    f32 = mybir.dt.float32
    f32r = mybir.dt.float32r
    Exp = mybir.ActivationFunctionType.Exp

    bf16 = mybir.dt.bfloat16
    nc = bacc.Bacc(
        "TRN2", target_bir_lowering=False, debug=False,
        enable_asserts=False, num_devices=8,
    )
    xh_d = nc.dram_tensor("xh", [S, D], bf16, kind="ExternalInput").ap()
    xl_d = nc.dram_tensor("xl", [S, D], bf16, kind="ExternalInput").ap()
    wq_d = nc.dram_tensor("wq", [D, COLS], f32, kind="ExternalInput").ap()
    wk_d = nc.dram_tensor("wk", [D, COLS], f32, kind="ExternalInput").ap()
    wv_d = nc.dram_tensor("wv", [D, COLS], f32, kind="ExternalInput").ap()
    wo_d = nc.dram_tensor("wo", [COLS, D], f32, kind="ExternalInput").ap()
    bq_d = nc.dram_tensor("bq", [COLS], f32, kind="ExternalInput").ap()
    bk_d = nc.dram_tensor("bk", [COLS], f32, kind="ExternalInput").ap()
    out_d = nc.dram_tensor("out_t", [D, S], f32, kind="ExternalOutput").ap()

    with tile.TileContext(nc) as tc:
        with (
            tc.tile_pool(name="const", bufs=1) as const,
            tc.tile_pool(name="wst", bufs=1) as wst,
            tc.tile_pool(name="wpool", bufs=1) as wpool,
            tc.tile_pool(name="persist", bufs=1) as persist,
            tc.tile_pool(name="xhl", bufs=1) as xhl,
            tc.tile_pool(name="xtp", bufs=2) as xtp,
            tc.tile_pool(name="exps", bufs=3) as exps,
            tc.tile_pool(name="stage", bufs=3) as stage,
            tc.tile_pool(name="outst", bufs=4) as outst,
            tc.tile_pool(name="ps_sc", bufs=1, space="PSUM") as ps_sc,
            tc.tile_pool(name="ps_acc", bufs=2, space="PSUM") as ps_acc,
            tc.tile_pool(name="ps_u", bufs=2, space="PSUM") as ps_u,
        ):
            ones32 = const.tile([P, VW], f32, tag="ones32")
            nc.vector.memset(ones32[:], 1.0)
            ones_r = const.tile([P, VW], f32r, tag="ones_r")
            nc.vector.tensor_copy(ones_r[:], ones32[:])

            # ---- weights: DMA fp32 -> convert to f32r on DVE ----
            def load_w(dram, shape_free, name):
                st = wst.tile([P, KC, shape_free], f32, tag="wstage", name="wstage")
                nc.sync.dma_start(st[:], dram.rearrange("(o p) f -> p o f", p=P))
                wr = wpool.tile([P, KC, shape_free], f32r, tag=f"w_{name}",
                                name=f"w_{name}")
                nc.vector.tensor_copy(wr[:], st[:])
                return wr

            wq_r = load_w(wq_d, COLS, "q")
            wk_r = load_w(wk_d, COLS, "k")
            wv_r = load_w(wv_d, COLS, "v")
            wo_st = wst.tile([P, 2, D], f32, tag="wstage", name="wostage")
            nc.sync.dma_start(wo_st[:], wo_d.rearrange("(o p) f -> p o f", p=P))
            wo_r = wpool.tile([P, 2, D], f32r, tag="w_o")
            nc.vector.tensor_copy(wo_r[:], wo_st[:])

            bq_sb = const.tile([P, 2], f32, tag="bq")
            nc.sync.dma_start(bq_sb[:], bq_d.rearrange("(o p) -> p o", p=P))
            bk_sb = const.tile([P, 2], f32, tag="bk")
            nc.sync.dma_start(bk_sb[:], bk_d.rearrange("(o p) -> p o", p=P))

            # persistent activations
            qT = persist.tile([P, 2, S], f32r, tag="qT")    # [qcol, tok]
            kT = persist.tile([P, 2, S], f32r, tag="kT")    # [kcol, tok]
            vt = persist.tile([P, NT, 4 * VW], f32r, tag="vt")  # [tok, h*(1|V)]
            oT = persist.tile([P, 2, S], f32r, tag="oT")    # [vdim, tok]

            # ones column (index 64 of each head's VW slice)
            vt_heads = vt[:].rearrange("p t (h c) -> p t h c", c=VW)
            nc.vector.tensor_copy(
                vt_heads[:, :, :, 64],
                ones32[:, :NT * 4].rearrange("p (t h) -> p t h", h=4),
            )

            # ---- phase 0/1: x transpose + QKV projections, per token block ----
            for j in range(NJ):
                xT = xtp.tile([P, KC, TT], f32r, tag="xT")
                xth = xhl.tile([P, KC, TT], bf16, tag="xth", name="xth")
                nc.sync.dma_start_transpose(xth[:], xh_d[bass.ts(j, TT), :])
                xtl = xhl.tile([P, KC, TT], bf16, tag="xtl", name="xtl")
                nc.sync.dma_start_transpose(xtl[:], xl_d[bass.ts(j, TT), :])
                nc.vector.tensor_tensor(
                    xT[:], xth[:], xtl[:], mybir.AluOpType.add
                )

                # Q^T, K^T: [qcol, tok] with bias
                for (wmat, bsb, dstT) in ((wq_r, bq_sb, qT), (wk_r, bk_sb, kT)):
                    for ct in range(2):
                        acc = ps_u.tile([P, TT], f32, tag="u", name="qk_acc")
                        for kc in range(KC):
                            nc.tensor.matmul(
                                acc[:], wmat[:, kc, bass.ts(ct, P)], xT[:, kc, :],
                                start=(kc == 0), stop=(kc == KC - 1),
                            )
                        nc.vector.tensor_scalar_add(
                            dstT[:, ct, bass.ts(j, TT)], acc[:], bsb[:, ct : ct + 1]
                        )

                # V: [tok, vcol]
                for ts in range(TT // P):
                    acc = ps_u.tile([P, COLS], f32, tag="u", name="v_acc")
                    for kc in range(KC):
                        nc.tensor.matmul(
                            acc[:], xT[:, kc, bass.ts(ts, P)], wv_r[:, kc, :],
                            start=(kc == 0), stop=(kc == KC - 1),
                        )
                    tt = 4 * j + ts
                    nc.vector.tensor_copy(
                        vt_heads[:, tt, :, 0:64],
                        acc[:].rearrange("p (h c) -> p h c", c=64),
                    )

            # shared scores PSUM tensor: 4 slots x [128, 512] = 4 banks
            big_sc = ps_sc.tile([P, 4, TT], f32, tag="sc")

            # ---- phase 2 + 3 interleaved over token blocks ----
            for j in range(NJ):
                for p in range(2):
                    o_ps = [
                        ps_acc.tile([VW, TT], f32, tag="acc", name=f"o_ps{i}")
                        for i in range(2)
                    ]
                    # software-pipelined emission: scores run 2 k-chunks ahead,
                    # AV trails exp by one, so PE always has ready work while
                    # ACT's ~1.1us exp latency is in flight.
                    def sc_emit(kc):
                        base = (2 * kc) % 4
                        for i in range(2):
                            lo, hi = 64 * i, 64 * i + 64
                            nc.tensor.matmul(
                                big_sc[:, base + i, :],
                                kT[lo:hi, p, bass.ts(kc, P)],
                                qT[lo:hi, p, bass.ts(j, TT)],
                                start=True, stop=True,
                            )

                    def av_emit(kc, ex):
                        for i in range(2):
                            h = 2 * p + i
                            nc.tensor.matmul(
                                o_ps[i][:],
                                vt[:, kc, bass.ds(VW * h, VW)],
                                ex[:, i, :],
                                start=(kc == 0), stop=(kc == NKT - 1),
                            )

                    sc_emit(0)
                    sc_emit(1)
                    prev = None
                    for kc in range(NKT):
                        base = (2 * kc) % 4
                        ex = exps.tile([P, 2, TT], f32r, tag="exp", name="ex")
                        nc.scalar.activation(
                            ex[:], big_sc[:, base : base + 2, :], Exp,
                            scale=0.125,
                        )
                        if prev is not None:
                            av_emit(kc - 1, prev)
                        if kc + 2 < NKT:
                            sc_emit(kc + 2)
                        prev = ex
                    av_emit(NKT - 1, prev)

                    # normalize both heads into O^T via recip/broadcast/mult/DMA
                    o32 = stage.tile([P, TT], f32r, tag="o32", name="o32")
                    for i in range(2):
                        # free o_ps quickly: one copy to SBUF, then normalize
                        osb = stage.tile([P, TT], f32r, tag="osb", name="osb")
                        nc.vector.tensor_copy(osb[0:VW, :], o_ps[i][:])
                        # broadcast the sums row via PE ones outer-product
                        rbc = ps_u.tile([64, TT], f32, tag="u", name="rbc")
                        nc.tensor.matmul(
                            rbc[:], ones_r[64:65, 0:64], osb[64:65, :],
                            start=True, stop=True,
                        )
                        rbs = stage.tile([64, TT], f32, tag="rbs", name="rbs")
                        nc.vector.reciprocal_approx_fast(rbs[:], rbc[:])
                        onrm = stage.tile([P, TT], f32r, tag="onrm", name="onrm")
                        nc.vector.tensor_tensor(
                            onrm[0:64, :], osb[0:64, :], rbs[:],
                            mybir.AluOpType.mult,
                        )
                        nc.sync.dma_start(
                            o32[bass.ds(64 * i, 64), :], onrm[0:64, :]
                        )
                    nc.vector.tensor_copy(oT[:, p, bass.ts(j, TT)], o32[:])

                # partial output projection for this token block
                for oc in range(D // P):
                    acc = ps_u.tile([P, TT], f32, tag="u", name="wo_acc")
                    for vc in range(2):
                        nc.tensor.matmul(
                            acc[:], wo_r[:, vc, bass.ts(oc, P)],
                            oT[:, vc, bass.ts(j, TT)],
                            start=(vc == 0), stop=(vc == 1),
                        )
                    st = outst.tile([P, TT], f32, tag="outst", name="outst")
                    nc.vector.tensor_copy(st[:], acc[:])
                    nc.sync.dma_start(out_d[bass.ts(oc, P), bass.ts(j, TT)], st[:])

    nc.compile()
    return nc


def make_in_maps(x, Wq, bq, Wk, bk, Wv, Wo):
    import ml_dtypes

    xh = [None, None]
    xl = [None, None]
    for b in range(B):
        hi = x[b].astype(ml_dtypes.bfloat16)
        lo = (x[b] - hi.astype(np.float32)).astype(ml_dtypes.bfloat16)
        xh[b], xl[b] = np.ascontiguousarray(hi), np.ascontiguousarray(lo)

    in_maps = []
    for c in range(8):
        b, g = divmod(c, 4)
        cs = slice(COLS * g, COLS * (g + 1))
        in_maps.append({
            "xh": xh[b],
            "xl": xl[b],
            "wq": np.ascontiguousarray(Wq[:, cs]),
            "wk": np.ascontiguousarray(Wk[:, cs]),
            "wv": np.ascontiguousarray(Wv[:, cs]),
            "wo": np.ascontiguousarray(Wo[cs, :]),
            "bq": np.ascontiguousarray(bq[cs]),
            "bk": np.ascontiguousarray(bk[cs]),
        })
    return in_maps


def kernel(x, Wq, bq, Wk, bk, Wv, bv, Wo, bo):
    from concourse import bass_utils

    x = np.asarray(x, dtype=np.float32)
    Wq = np.asarray(Wq, dtype=np.float32)
    Wk = np.asarray(Wk, dtype=np.float32)
    Wv = np.asarray(Wv, dtype=np.float32)
    Wo = np.asarray(Wo, dtype=np.float32)
    bq = np.asarray(bq, dtype=np.float32)
    bk = np.asarray(bk, dtype=np.float32)
    bv = np.asarray(bv, dtype=np.float32)
    bo = np.asarray(bo, dtype=np.float32)

    if "nc" not in _CACHE:
        _CACHE["nc"] = _build()
    nc = _CACHE["nc"]

    in_maps = make_in_maps(x, Wq, bq, Wk, bk, Wv, Wo)
    res = bass_utils.run_bass_kernel_spmd(nc, in_maps, core_ids=list(range(8)))

    out = np.zeros((B, S, D), dtype=np.float32)
    for c in range(8):
        out[c // 4] += res.results[c]["out_t"].T
    out += bo + bv @ Wo
    return out



# revision 6
# speedup vs baseline: 1.0011x; 1.0011x over previous
"""Multi-head attention (B=2, S=2048, D=1024, H=16, dk=64) on 8 Trainium2
NeuronCores via Bass/Tile.

Sharding: core c handles batch b = c//4 and head-group g = c%4 (4 heads,
256 qkv columns).  Each core computes its QKV projection slices, 4 heads of
attention, and a partial output projection against its 256-row slice of Wo.
The host sums the 4 partial outputs per batch and folds in bo and bv@Wo.

v3 design notes (vs v2 baseline at ~373us):
- All operands bf16 (x, Wq/Wk/Wv/Wo, qT/kT/vt/ex/oT); PSUM accumulates
  fp32.  Errors average out over the large contractions; halves DMA and
  SBUF traffic and enables fast-weight-load on 128-col LDWEIGHTS.
- bk dropped entirely: it shifts every score of a (q,head) row by the same
  constant, which softmax cancels exactly.
- exp was the phase-B bottleneck (ACT = 1 elem/cyc/lane -> 142us > PE work,
  starving the PE and triggering HAM 4/8 down-throttle for ~60% of the
  kernel).  Now split: kc % 4 == 3 computed on DVE with a two-term
  product-form Schraudolph (exp(s) ~ bitcast(A/2*s+B) * bitcast(A/2*s+B+64),
  opposite sawtooth phases; global sqrt(2) scale cancels in softmax), the
  other 3/4 on ACT (exact).  Both engines land ~95% busy under the PE pace.
- Deeper pipeline: scores run 2 kc ahead in a 4-bank PSUM rotation, exp one
  ahead, AV trails; PE never idles so HAM stays at 8/8 (2.4 GHz).
- Normalization: denominators via the ones-column of the AV lhsT; recip +
  broadcast as in v2 but the normalized halves DMA straight into oT (bf16).
"""

import numpy as np

P = 128
B, S, D = 2, 2048, 1024
H, DK = 16, 64
COLS = 256          # qkv columns per core (4 heads)
KC = D // P         # 8 contraction chunks for the projections
TT = 512            # token block (matmul free dim)
NJ = S // TT        # 4 token blocks
NT = S // P         # 16 token tiles
NKT = S // P        # 16 key tiles
VW = 65             # per-head AV lhsT width: 64 v-dims + ones column

# two-term product Schraudolph for exp(0.125*s) on bf16 bit patterns:
#   exp(x) = 2^(x*log2e); bf16 bits b encode 2^((b-16256)/128) up to the
#   mantissa-vs-log sawtooth.  t = 0.125*s*log2e*128; use half-scale terms
#   t/2 + B and t/2 + B + 64 whose sawtooth phases are opposite; the product
#   restores t and cancels most of the sawtooth.  The leftover sqrt(2)
#   factor is uniform and cancels in softmax.
SCHR_A = 0.125 * 1.4426950408889634 * 64.0   # = 0.125*log2(e)*128/2
# B1+B2 = 2*16256 - 64 - 15 : the -64 removes the sqrt(2) of the +64 phase
# offset, the -15 bits (~2*128*E[sawtooth]) zero the mean vs ACT's exact exp
# so the two engines' outputs mix consistently inside one softmax.
SCHR_B1 = 16216.5
SCHR_B2 = 16280.5

_CACHE = {}


def _build():
    import concourse.bass as bass
    import concourse.tile as tile
    from concourse import bacc, mybir

    f32 = mybir.dt.float32
    f32r = mybir.dt.float32r
    bf16 = mybir.dt.bfloat16
    i16 = mybir.dt.int16
    Exp = mybir.ActivationFunctionType.Exp
    MUL = mybir.AluOpType.mult
    ADD = mybir.AluOpType.add

    nc = bacc.Bacc(
        "TRN2", target_bir_lowering=False, debug=False,
        enable_asserts=False, num_devices=8,
    )
    x_d = nc.dram_tensor("x", [S, D], bf16, kind="ExternalInput").ap()
    wq_d = nc.dram_tensor("wq", [D, COLS], bf16, kind="ExternalInput").ap()
    wk_d = nc.dram_tensor("wk", [D, COLS], bf16, kind="ExternalInput").ap()
    wv_d = nc.dram_tensor("wv", [D, COLS], bf16, kind="ExternalInput").ap()
    wo_d = nc.dram_tensor("wo", [COLS, D], bf16, kind="ExternalInput").ap()
    bq_d = nc.dram_tensor("bq", [COLS], f32, kind="ExternalInput").ap()
    out_d = nc.dram_tensor("out_t", [D, S], f32, kind="ExternalOutput").ap()

    with tile.TileContext(nc) as tc:
        with (
            tc.tile_pool(name="const", bufs=1) as const,
            tc.tile_pool(name="wpool", bufs=1) as wpool,
            tc.tile_pool(name="persist", bufs=1) as persist,
            tc.tile_pool(name="xtp", bufs=2) as xtp,
            tc.tile_pool(name="exps", bufs=4) as exps,
            tc.tile_pool(name="schr", bufs=2) as schr,
            tc.tile_pool(name="stage", bufs=2) as stage,
            tc.tile_pool(name="outst", bufs=4) as outst,
            tc.tile_pool(name="ps_sc", bufs=1, space="PSUM") as ps_sc,
            tc.tile_pool(name="ps_acc", bufs=2, space="PSUM") as ps_acc,
            tc.tile_pool(name="ps_u", bufs=2, space="PSUM") as ps_u,
        ):
            # ---- input DMAs first: x block 0 transpose, then weights ----
            xTs = {}
            xTs[0] = xtp.tile([P, KC, TT], bf16, tag="xT", name="xT0")
            nc.sync.dma_start_transpose(xTs[0][:], x_d[bass.ts(0, TT), :])

            wq_sb = wpool.tile([P, KC, COLS], bf16, tag="wq")
            nc.sync.dma_start(wq_sb[:], wq_d.rearrange("(o p) f -> p o f", p=P))
            wk_sb = wpool.tile([P, KC, COLS], bf16, tag="wk")
            nc.sync.dma_start(wk_sb[:], wk_d.rearrange("(o p) f -> p o f", p=P))
            wv_sb = wpool.tile([P, KC, COLS], bf16, tag="wv")
            nc.sync.dma_start(wv_sb[:], wv_d.rearrange("(o p) f -> p o f", p=P))
            wo_sb = wpool.tile([P, 2, D], bf16, tag="wo")
            nc.sync.dma_start(wo_sb[:], wo_d.rearrange("(o p) f -> p o f", p=P))
            bq_sb = const.tile([P, 2], f32, tag="bq")
            nc.sync.dma_start(bq_sb[:], bq_d.rearrange("(o p) -> p o", p=P))

            # ones: f32r [P, VW] for the norm broadcast; bf16 row for vt
            ones32 = const.tile([P, VW], f32, tag="ones32")
            nc.vector.memset(ones32[:], 1.0)
            ones_r = const.tile([P, VW], f32r, tag="ones_r")
            nc.vector.tensor_copy(ones_r[:], ones32[:])
            ones_bf = const.tile([P, NT * 4], bf16, tag="ones_bf")
            nc.vector.memset(ones_bf[:], 1.0)

            # preload the Exp table while DMAs run
            dummy = const.tile([P, 1], f32, tag="dummy")
            nc.scalar.activation(dummy[:], ones32[:, 0:1], Exp, scale=1.0)

            # persistent activations (all bf16)
            qT = persist.tile([P, 2, S], bf16, tag="qT")    # [qcol, tok]
            kT = persist.tile([P, 2, S], bf16, tag="kT")    # [kcol, tok]
            vt = persist.tile([P, NT, 4 * VW], bf16, tag="vt")  # [tok, h*(V|1)]
            oT = persist.tile([P, 2, S], bf16, tag="oT")    # [vdim, tok]

            vt_heads = vt[:].rearrange("p t (h c) -> p t h c", c=VW)
            nc.vector.tensor_copy(
                vt_heads[:, :, :, 64],
                ones_bf[:].rearrange("p (t h) -> p t h", h=4),
            )

            # ---- phase A: QKV projections per token block ----
            for j in range(NJ):
                if j + 1 < NJ:
                    xTs[j + 1] = xtp.tile([P, KC, TT], bf16, tag="xT",
                                          name=f"xT{j+1}")
                    nc.sync.dma_start_transpose(
                        xTs[j + 1][:], x_d[bass.ts(j + 1, TT), :]
                    )
                xT = xTs.pop(j)

                # Q^T (with bias), K^T (no bias: softmax cancels bk)
                for (wmat, dstT, bias) in (
                    (wq_sb, qT, bq_sb), (wk_sb, kT, None),
                ):
                    for ct in range(2):
                        acc = ps_u.tile([P, TT], f32, tag="u", name="qk_acc")
                        for kc in range(KC):
                            nc.tensor.matmul(
                                acc[:], wmat[:, kc, bass.ts(ct, P)], xT[:, kc, :],
                                start=(kc == 0), stop=(kc == KC - 1),
                            )
                        if bias is not None:
                            nc.vector.tensor_scalar_add(
                                dstT[:, ct, bass.ts(j, TT)], acc[:],
                                bias[:, ct : ct + 1],
                            )
                        else:
                            nc.vector.tensor_copy(
                                dstT[:, ct, bass.ts(j, TT)], acc[:]
                            )

                # V: [tok, vcol]
                for ts4 in range(TT // P):
                    acc = ps_u.tile([P, COLS], f32, tag="u", name="v_acc")
                    for kc in range(KC):
                        nc.tensor.matmul(
                            acc[:], xT[:, kc, bass.ts(ts4, P)], wv_sb[:, kc, :],
                            start=(kc == 0), stop=(kc == KC - 1),
                        )
                    tt = 4 * j + ts4
                    nc.vector.tensor_copy(
                        vt_heads[:, tt, :, 0:64],
                        acc[:].rearrange("p (h c) -> p h c", c=64),
                    )

            # shared scores PSUM: 4 banks, pair-rotated (2 kc in flight)
            big_sc = ps_sc.tile([P, 4, TT], f32, tag="sc")

            # ---- phase B: attention + output projection per block ----
            for j in range(NJ):
                for p in range(2):
                    o_ps = [
                        ps_acc.tile([VW, TT], f32, tag="acc", name=f"o_ps{i}")
                        for i in range(2)
                    ]

                    def sc_pair(kc):
                        base = (2 * kc) % 4
                        for i in range(2):
                            lo = 64 * i
                            nc.tensor.matmul(
                                big_sc[:, base + i, :],
                                kT[lo : lo + 64, p, bass.ts(kc, P)],
                                qT[lo : lo + 64, p, bass.ts(j, TT)],
                                start=True, stop=True,
                            )

                    def exp_emit(kc):
                        base = (2 * kc) % 4
                        ex = exps.tile([P, 2, TT], bf16, tag="ex", name="ex")
                        if kc % 4 != 3:
                            nc.scalar.activation(
                                ex[:], big_sc[:, base : base + 2, :], Exp,
                                scale=0.125,
                            )
                        else:
                            # ACT evacuates the banks to SBUF fp32 (frees the
                            # banks fast + lets the TS ops run in 2x DVE mode,
                            # which a PSUM fp32 source would forbid)
                            scf = schr.tile([P, 2, TT], f32, tag="scf",
                                            name="scf")
                            nc.scalar.copy(scf[:], big_sc[:, base : base + 2, :])
                            t1 = schr.tile([P, 2, TT], i16, tag="t1", name="t1")
                            t2 = schr.tile([P, 2, TT], i16, tag="t2", name="t2")
                            nc.vector.tensor_scalar(
                                t1[:], scf[:], SCHR_A, SCHR_B1, MUL, ADD,
                            )
                            nc.vector.tensor_scalar(
                                t2[:], scf[:], SCHR_A, SCHR_B2, MUL, ADD,
                            )
                            nc.vector.tensor_tensor(
                                ex[:], t1[:].bitcast(bf16), t2[:].bitcast(bf16),
                                MUL,
                            )
                        return ex

                    def av_pair(kc, ex):
                        for i in range(2):
                            h = 2 * p + i
                            nc.tensor.matmul(
                                o_ps[i][:],
                                vt[:, kc, bass.ds(VW * h, VW)],
                                ex[:, i, :],
                                start=(kc == 0), stop=(kc == NKT - 1),
                            )

                    # exp chases each sc_pair immediately (2-iteration lead
                    # before its banks are rewritten by sc_pair(kc+4... mod 4))
                    sc_pair(0)
                    exq = {0: exp_emit(0)}
                    sc_pair(1)
                    exq[1] = exp_emit(1)
                    for kc in range(NKT):
                        av_pair(kc, exq.pop(kc))
                        if kc + 2 < NKT:
                            sc_pair(kc + 2)
                            exq[kc + 2] = exp_emit(kc + 2)

                    # normalize both heads straight into oT (bf16)
                    for i in range(2):
                        osb = stage.tile([P, TT], f32r, tag="osb", name="osb")
                        if i == 0:
                            nc.scalar.copy(osb[0:VW, :], o_ps[i][:])
                        else:
                            nc.vector.tensor_copy(osb[0:VW, :], o_ps[i][:])
                        rbc = ps_u.tile([64, TT], f32, tag="u", name="rbc")
                        nc.tensor.matmul(
                            rbc[:], ones_r[64:65, 0:64], osb[64:65, :],
                            start=True, stop=True,
                        )
                        rbs = stage.tile([64, TT], f32, tag="rbs", name="rbs")
                        nc.vector.reciprocal_approx_fast(rbs[:], rbc[:])
                        onrm = stage.tile([64, TT], bf16, tag="onrm",
                                          name="onrm")
                        nc.vector.tensor_tensor(
                            onrm[:], osb[0:64, :], rbs[:], MUL,
                        )
                        nc.sync.dma_start(
                            oT[bass.ds(64 * i, 64), p, bass.ts(j, TT)], onrm[:]
                        )

                # partial output projection for this token block
                for oc in range(D // P):
                    acc = ps_u.tile([P, TT], f32, tag="u", name="wo_acc")
                    for vc in range(2):
                        nc.tensor.matmul(
                            acc[:], wo_sb[:, vc, bass.ts(oc, P)],
                            oT[:, vc, bass.ts(j, TT)],
                            start=(vc == 0), stop=(vc == 1),
                        )
                    st = outst.tile([P, TT], f32, tag="outst", name="outst")
                    if oc % 2 == 0:
                        nc.scalar.copy(st[:], acc[:])
                    else:
                        nc.vector.tensor_copy(st[:], acc[:])
                    nc.sync.dma_start(out_d[bass.ts(oc, P), bass.ts(j, TT)], st[:])

    nc.compile()
    return nc


def make_in_maps(x, Wq, bq, Wk, bk, Wv, Wo):
    import ml_dtypes

    bf = ml_dtypes.bfloat16
    xb = [np.ascontiguousarray(x[b].astype(bf)) for b in range(B)]
    wqb = Wq.astype(bf)
    wkb = Wk.astype(bf)
    wvb = Wv.astype(bf)
    wob = Wo.astype(bf)

    in_maps = []
    for c in range(8):
        b, g = divmod(c, 4)
        cs = slice(COLS * g, COLS * (g + 1))
        in_maps.append({
            "x": xb[b],
            "wq": np.ascontiguousarray(wqb[:, cs]),
            "wk": np.ascontiguousarray(wkb[:, cs]),
            "wv": np.ascontiguousarray(wvb[:, cs]),
            "wo": np.ascontiguousarray(wob[cs, :]),
            "bq": np.ascontiguousarray(bq[cs].astype(np.float32)),
        })
    return in_maps


def kernel(x, Wq, bq, Wk, bk, Wv, bv, Wo, bo):
    from concourse import bass_utils

    x = np.asarray(x, dtype=np.float32)
    Wq = np.asarray(Wq, dtype=np.float32)
    Wk = np.asarray(Wk, dtype=np.float32)
    Wv = np.asarray(Wv, dtype=np.float32)
    Wo = np.asarray(Wo, dtype=np.float32)
    bq = np.asarray(bq, dtype=np.float32)
    bv = np.asarray(bv, dtype=np.float32)
    bo = np.asarray(bo, dtype=np.float32)

    if "nc" not in _CACHE:
        _CACHE["nc"] = _build()
    nc = _CACHE["nc"]

    in_maps = make_in_maps(x, Wq, bq, Wk, bk, Wv, Wo)
    res = bass_utils.run_bass_kernel_spmd(nc, in_maps, core_ids=list(range(8)))

    out = np.zeros((B, S, D), dtype=np.float32)
    for c in range(8):
        out[c // 4] += res.results[c]["out_t"].T
    out += bo + bv @ Wo
    return out


# revision 9
# speedup vs baseline: 1.1810x; 1.1798x over previous
"""Multi-head attention (B=2, S=2048, D=1024, H=16, dk=64) on 8 Trainium2
NeuronCores via Bass/Tile.

Sharding: core c handles batch b = c//4 and head-group g = c%4 (4 heads,
256 qkv columns).  Each core computes its QKV projection slices, 4 heads of
attention, and a partial output projection against its 256-row slice of Wo.
The host sums the 4 partial outputs per batch and folds in bo and bv@Wo.

v3 design notes (vs v2 baseline at ~373us):
- All operands bf16 (x, Wq/Wk/Wv/Wo, qT/kT/vt/ex/oT); PSUM accumulates
  fp32.  Errors average out over the large contractions; halves DMA and
  SBUF traffic and enables fast-weight-load on 128-col LDWEIGHTS.
- bk dropped entirely: it shifts every score of a (q,head) row by the same
  constant, which softmax cancels exactly.
- exp was the phase-B bottleneck (ACT = 1 elem/cyc/lane -> 142us > PE work,
  starving the PE and triggering HAM 4/8 down-throttle for ~60% of the
  kernel).  Now split: kc % 4 == 3 computed on DVE with a two-term
  product-form Schraudolph (exp(s) ~ bitcast(A/2*s+B) * bitcast(A/2*s+B+64),
  opposite sawtooth phases; global sqrt(2) scale cancels in softmax), the
  other 3/4 on ACT (exact).  Both engines land ~95% busy under the PE pace.
- Deeper pipeline: scores run 2 kc ahead in a 4-bank PSUM rotation, exp one
  ahead, AV trails; PE never idles so HAM stays at 8/8 (2.4 GHz).
- Normalization: denominators via the ones-column of the AV lhsT; recip +
  broadcast as in v2 but the normalized halves DMA straight into oT (bf16).
"""

import numpy as np

P = 128
B, S, D = 2, 2048, 1024
H, DK = 16, 64
COLS = 256          # qkv columns per core (4 heads)
KC = D // P         # 8 contraction chunks for the projections
TT = 512            # token block (matmul free dim)
NJ = S // TT        # 4 token blocks
NT = S // P         # 16 token tiles
NKT = S // P        # 16 key tiles
VW = 65             # per-head AV lhsT width: 64 v-dims + ones column

# two-term product Schraudolph for exp(0.125*s) on bf16 bit patterns:
#   exp(x) = 2^(x*log2e); bf16 bits b encode 2^((b-16256)/128) up to the
#   mantissa-vs-log sawtooth.  t = 0.125*s*log2e*128; use half-scale terms
#   t/2 + B and t/2 + B + 64 whose sawtooth phases are opposite; the product
#   restores t and cancels most of the sawtooth.  The leftover sqrt(2)
#   factor is uniform and cancels in softmax.
SCHR_A = 0.125 * 1.4426950408889634 * 128.0  # = 0.125*log2(e)*128
# 16256 = 127<<7 (bf16 bits of 1.0); -7.25 bits zero the mean sawtooth error
# against ACT's exact exp so both engines mix consistently in one softmax.
SCHR_B = 16248.75
DVE_KCS = frozenset({3, 7, 11, 15})

_CACHE = {}


def _build():
    import concourse.bass as bass
    import concourse.tile as tile
    from concourse import bacc, mybir

    f32 = mybir.dt.float32
    f32r = mybir.dt.float32r
    bf16 = mybir.dt.bfloat16
    i16 = mybir.dt.int16
    Exp = mybir.ActivationFunctionType.Exp
    MUL = mybir.AluOpType.mult
    ADD = mybir.AluOpType.add

    nc = bacc.Bacc(
        "TRN2", target_bir_lowering=False, debug=False,
        enable_asserts=False, num_devices=8,
    )
    x_d = nc.dram_tensor("x", [S, D], bf16, kind="ExternalInput").ap()
    wq_d = nc.dram_tensor("wq", [D, COLS], bf16, kind="ExternalInput").ap()
    wk_d = nc.dram_tensor("wk", [D, COLS], bf16, kind="ExternalInput").ap()
    wv_d = nc.dram_tensor("wv", [D, COLS], bf16, kind="ExternalInput").ap()
    wo_d = nc.dram_tensor("wo", [COLS, D], bf16, kind="ExternalInput").ap()
    bq_d = nc.dram_tensor("bq", [COLS], f32, kind="ExternalInput").ap()
    out_d = nc.dram_tensor("out_t", [D, S], f32, kind="ExternalOutput").ap()

    with tile.TileContext(nc) as tc:
        with (
            tc.tile_pool(name="const", bufs=1) as const,
            tc.tile_pool(name="wpool", bufs=1) as wpool,
            tc.tile_pool(name="persist", bufs=1) as persist,
            tc.tile_pool(name="xtp", bufs=2) as xtp,
            tc.tile_pool(name="exps", bufs=4) as exps,
            tc.tile_pool(name="schr", bufs=2) as schr,
            tc.tile_pool(name="stage", bufs=2) as stage,
            tc.tile_pool(name="outst", bufs=4) as outst,
            tc.tile_pool(name="ps_sc", bufs=1, space="PSUM") as ps_sc,
            tc.tile_pool(name="ps_acc", bufs=2, space="PSUM") as ps_acc,
            tc.tile_pool(name="ps_u", bufs=2, space="PSUM") as ps_u,
        ):
            # ---- input DMAs first: x block 0 transpose, then weights ----
            xTs = {}
            xTs[0] = xtp.tile([P, KC, TT], bf16, tag="xT", name="xT0")
            nc.sync.dma_start_transpose(xTs[0][:], x_d[bass.ts(0, TT), :])

            wq_sb = wpool.tile([P, KC, COLS], bf16, tag="wq")
            nc.sync.dma_start(wq_sb[:], wq_d.rearrange("(o p) f -> p o f", p=P))
            wk_sb = wpool.tile([P, KC, COLS], bf16, tag="wk")
            nc.sync.dma_start(wk_sb[:], wk_d.rearrange("(o p) f -> p o f", p=P))
            wv_sb = wpool.tile([P, KC, COLS], bf16, tag="wv")
            nc.sync.dma_start(wv_sb[:], wv_d.rearrange("(o p) f -> p o f", p=P))
            wo_sb = wpool.tile([P, 2, D], bf16, tag="wo")
            nc.sync.dma_start(wo_sb[:], wo_d.rearrange("(o p) f -> p o f", p=P))
            bq_sb = const.tile([P, 2], f32, tag="bq")
            nc.sync.dma_start(bq_sb[:], bq_d.rearrange("(o p) -> p o", p=P))

            # ones: f32r [P, VW] for the norm broadcast; bf16 row for vt
            ones32 = const.tile([P, VW], f32, tag="ones32")
            nc.vector.memset(ones32[:], 1.0)
            ones_r = const.tile([P, VW], f32r, tag="ones_r")
            nc.vector.tensor_copy(ones_r[:], ones32[:])
            ones_bf = const.tile([P, NT * 4], bf16, tag="ones_bf")
            nc.vector.memset(ones_bf[:], 1.0)

            # preload the Exp table while DMAs run
            dummy = const.tile([P, 1], f32, tag="dummy")
            nc.scalar.activation(dummy[:], ones32[:, 0:1], Exp, scale=1.0)

            # persistent activations (all bf16)
            qT = persist.tile([P, 2, S], bf16, tag="qT")    # [qcol, tok]
            kT = persist.tile([P, 2, S], bf16, tag="kT")    # [kcol, tok]
            vt = persist.tile([P, NT, 4 * VW], bf16, tag="vt")  # [tok, h*(V|1)]
            oT = persist.tile([P, 2, S], bf16, tag="oT")    # [vdim, tok]

            vt_heads = vt[:].rearrange("p t (h c) -> p t h c", c=VW)
            nc.vector.tensor_copy(
                vt_heads[:, :, :, 64],
                ones_bf[:].rearrange("p (t h) -> p t h", h=4),
            )

            # ---- phase A: QKV projections per token block ----
            for j in range(NJ):
                if j + 1 < NJ:
                    xTs[j + 1] = xtp.tile([P, KC, TT], bf16, tag="xT",
                                          name=f"xT{j+1}")
                    nc.sync.dma_start_transpose(
                        xTs[j + 1][:], x_d[bass.ts(j + 1, TT), :]
                    )
                xT = xTs.pop(j)

                # Q^T (with bias), K^T (no bias: softmax cancels bk)
                for (wmat, dstT, bias) in (
                    (wq_sb, qT, bq_sb), (wk_sb, kT, None),
                ):
                    for ct in range(2):
                        acc = ps_u.tile([P, TT], f32, tag="u", name="qk_acc")
                        for kc in range(KC):
                            nc.tensor.matmul(
                                acc[:], wmat[:, kc, bass.ts(ct, P)], xT[:, kc, :],
                                start=(kc == 0), stop=(kc == KC - 1),
                            )
                        if bias is not None:
                            nc.vector.tensor_scalar_add(
                                dstT[:, ct, bass.ts(j, TT)], acc[:],
                                bias[:, ct : ct + 1],
                            )
                        else:
                            nc.vector.tensor_copy(
                                dstT[:, ct, bass.ts(j, TT)], acc[:]
                            )

                # V: [tok, vcol]
                for ts4 in range(TT // P):
                    acc = ps_u.tile([P, COLS], f32, tag="u", name="v_acc")
                    for kc in range(KC):
                        nc.tensor.matmul(
                            acc[:], xT[:, kc, bass.ts(ts4, P)], wv_sb[:, kc, :],
                            start=(kc == 0), stop=(kc == KC - 1),
                        )
                    tt = 4 * j + ts4
                    nc.vector.tensor_copy(
                        vt_heads[:, tt, :, 0:64],
                        acc[:].rearrange("p (h c) -> p h c", c=64),
                    )

            # shared scores PSUM: 4 banks, pair-rotated (2 kc in flight)
            big_sc = ps_sc.tile([P, 4, TT], f32, tag="sc")

            # ---- phase B: attention + output projection per block ----
            for j in range(NJ):
                for p in range(2):
                    o_ps = [
                        ps_acc.tile([VW, TT], f32, tag="acc", name=f"o_ps{i}")
                        for i in range(2)
                    ]

                    def sc_pair(kc):
                        base = (2 * kc) % 4
                        for i in range(2):
                            lo = 64 * i
                            nc.tensor.matmul(
                                big_sc[:, base + i, :],
                                kT[lo : lo + 64, p, bass.ts(kc, P)],
                                qT[lo : lo + 64, p, bass.ts(j, TT)],
                                start=True, stop=True,
                            )

                    def exp_emit(kc):
                        base = (2 * kc) % 4
                        ex = exps.tile([P, 2, TT], bf16, tag="ex", name="ex")
                        if kc not in DVE_KCS:
                            nc.scalar.activation(
                                ex[:], big_sc[:, base : base + 2, :], Exp,
                                scale=0.125,
                            )
                        else:
                            # Schraudolph on DVE: one tensor_scalar writes the
                            # bf16 bit pattern of exp(0.125*s) as int16
                            nc.vector.tensor_scalar(
                                ex[:].bitcast(i16),
                                big_sc[:, base : base + 2, :],
                                SCHR_A, SCHR_B, MUL, ADD,
                            )
                        return ex

                    def av_pair(kc, ex):
                        for i in range(2):
                            h = 2 * p + i
                            nc.tensor.matmul(
                                o_ps[i][:],
                                vt[:, kc, bass.ds(VW * h, VW)],
                                ex[:, i, :],
                                start=(kc == 0), stop=(kc == NKT - 1),
                            )

                    # exp chases each sc_pair immediately (2-iteration lead
                    # before its banks are rewritten by sc_pair(kc+4... mod 4))
                    sc_pair(0)
                    exq = {0: exp_emit(0)}
                    sc_pair(1)
                    exq[1] = exp_emit(1)
                    for kc in range(NKT):
                        av_pair(kc, exq.pop(kc))
                        if kc + 2 < NKT:
                            sc_pair(kc + 2)
                            exq[kc + 2] = exp_emit(kc + 2)

                    # normalize both heads straight into oT (bf16)
                    for i in range(2):
                        osb = stage.tile([P, TT], f32r, tag="osb", name="osb")
                        nc.vector.tensor_copy(osb[0:VW, :], o_ps[i][:])
                        rbc = ps_u.tile([64, TT], f32, tag="u", name="rbc")
                        nc.tensor.matmul(
                            rbc[:], ones_r[64:65, 0:64], osb[64:65, :],
                            start=True, stop=True,
                        )
                        rbs = stage.tile([64, TT], f32, tag="rbs", name="rbs")
                        nc.vector.reciprocal_approx_fast(rbs[:], rbc[:])
                        onrm = stage.tile([64, TT], bf16, tag="onrm",
                                          name="onrm")
                        nc.vector.tensor_tensor(
                            onrm[:], osb[0:64, :], rbs[:], MUL,
                        )
                        nc.sync.dma_start(
                            oT[bass.ds(64 * i, 64), p, bass.ts(j, TT)], onrm[:]
                        )

                # partial output projection for this token block
                for oc in range(D // P):
                    acc = ps_u.tile([P, TT], f32, tag="u", name="wo_acc")
                    for vc in range(2):
                        nc.tensor.matmul(
                            acc[:], wo_sb[:, vc, bass.ts(oc, P)],
                            oT[:, vc, bass.ts(j, TT)],
                            start=(vc == 0), stop=(vc == 1),
                        )
                    st = outst.tile([P, TT], f32, tag="outst", name="outst")
                    if oc % 2 == 0:
                        nc.scalar.copy(st[:], acc[:])
                    else:
                        nc.vector.tensor_copy(st[:], acc[:])
                    nc.sync.dma_start(out_d[bass.ts(oc, P), bass.ts(j, TT)], st[:])

    nc.compile()
    return nc


def make_in_maps(x, Wq, bq, Wk, bk, Wv, Wo):
    import ml_dtypes

    bf = ml_dtypes.bfloat16
    xb = [np.ascontiguousarray(x[b].astype(bf)) for b in range(B)]
    wqb = Wq.astype(bf)
    wkb = Wk.astype(bf)
    wvb = Wv.astype(bf)
    wob = Wo.astype(bf)

    in_maps = []
    for c in range(8):
        b, g = divmod(c, 4)
        cs = slice(COLS * g, COLS * (g + 1))
        in_maps.append({
            "x": xb[b],
            "wq": np.ascontiguousarray(wqb[:, cs]),
            "wk": np.ascontiguousarray(wkb[:, cs]),
            "wv": np.ascontiguousarray(wvb[:, cs]),
            "wo": np.ascontiguousarray(wob[cs, :]),
            "bq": np.ascontiguousarray(bq[cs].astype(np.float32)),
        })
    return in_maps


def kernel(x, Wq, bq, Wk, bk, Wv, bv, Wo, bo):
    from concourse import bass_utils

    x = np.asarray(x, dtype=np.float32)
    Wq = np.asarray(Wq, dtype=np.float32)
    Wk = np.asarray(Wk, dtype=np.float32)
    Wv = np.asarray(Wv, dtype=np.float32)
    Wo = np.asarray(Wo, dtype=np.float32)
    bq = np.asarray(bq, dtype=np.float32)
    bv = np.asarray(bv, dtype=np.float32)
    bo = np.asarray(bo, dtype=np.float32)

    if "nc" not in _CACHE:
        _CACHE["nc"] = _build()
    nc = _CACHE["nc"]

    in_maps = make_in_maps(x, Wq, bq, Wk, bk, Wv, Wo)
    res = bass_utils.run_bass_kernel_spmd(nc, in_maps, core_ids=list(range(8)))

    out = np.zeros((B, S, D), dtype=np.float32)
    for c in range(8):
        out[c // 4] += res.results[c]["out_t"].T
    out += bo + bv @ Wo
    return out
